# revision 4
# baseline (speedup 1.0000x reference)
"""DeepSeekMoE block (router + top-2-of-8 experts + shared expert) on 8 TRN2 cores.

Strategy: data-parallel over tokens. Each of the 8 cores processes T/8 = 512
tokens end-to-end: fp32 router + top-2 gates on device, dense (masked) expert
SwiGLU in bf16 with fp32 accumulation, shared expert SwiGLU in bf16, all
matmuls feature-major (contraction dim on partitions).

Self-contained: builds + compiles the Bass program on first call, runs it via
run_bass_kernel_spmd on cores 0-7, and reassembles full outputs.
"""

import os
import sys
import types

import numpy as np
import ml_dtypes

# ---------------------------------------------------------------- NTFF shim
# The image's antenv package lacks axon_hooks, so concourse cannot register
# the NTFF profiling hook on its own. Provide it so trace=True works.
def _install_ntff_shim():
    if "antenv.axon_hooks" in sys.modules:
        return
    mod = types.ModuleType("antenv.axon_hooks")
    mod._hook = None
    mod.set_axon_ntff_profile_hook = lambda h: setattr(mod, "_hook", h)
    mod.get_axon_ntff_profile_hook = lambda: mod._hook
    try:
        import antenv

        sys.modules["antenv.axon_hooks"] = mod
        antenv.axon_hooks = mod
        from trn_agent_boot.trn_boot import _ntff_profile_via_ctypes

        hook = _ntff_profile_via_ctypes("/opt/axon/libaxon_pjrt.so")
        if hook is not None:
            mod.set_axon_ntff_profile_hook(hook)
    except Exception:
        pass


_install_ntff_shim()

import concourse.bacc as bacc
import concourse.bass as bass
import concourse.mybir as mybir
import concourse.tile as tile
from concourse.bass_utils import run_bass_kernel_spmd
from concourse.masks import make_identity

# ---------------------------------------------------------------- constants
B, S, D = 2, 2048, 1024
E, TOPK, I = 8, 2, 512
SI = 2048
T = B * S            # 4096 tokens
NCORES = 8
TC = T // NCORES     # 512 tokens per core
P = 128
KD = D // P          # 8  d-tiles (contraction for gate/up/router)
TT = TC // P         # 4  token tiles
IT = I // P          # 4  i-tiles per expert
DT = D // P          # 8  d-tiles (output of down proj)
SIT = SI // P        # 16 si-tiles
SQ = 4               # shared expert processed in 4 si-groups of 4 tiles

F32 = mybir.dt.float32
BF16 = mybir.dt.bfloat16

_COMPILED = None
LAST_RESULT = None


def _build_program():
    nc = bacc.Bacc(
        "TRN2", target_bir_lowering=False, debug=False, enable_asserts=False
    )
    dr = {}
    dr["x"] = nc.dram_tensor("x", [TC, D], F32, kind="ExternalInput").ap()
    dr["wr"] = nc.dram_tensor("wr", [D, E], F32, kind="ExternalInput").ap()
    dr["wg"] = nc.dram_tensor("wg", [E, D, I], BF16, kind="ExternalInput").ap()
    dr["wu"] = nc.dram_tensor("wu", [E, D, I], BF16, kind="ExternalInput").ap()
    dr["wd"] = nc.dram_tensor("wd", [E, I, D], BF16, kind="ExternalInput").ap()
    dr["wgs"] = nc.dram_tensor("wgs", [D, SI], BF16, kind="ExternalInput").ap()
    dr["wus"] = nc.dram_tensor("wus", [D, SI], BF16, kind="ExternalInput").ap()
    dr["wds"] = nc.dram_tensor("wds", [SI, D], BF16, kind="ExternalInput").ap()
    dr["y"] = nc.dram_tensor("y", [TC, D], F32, kind="ExternalOutput").ap()
    dr["logits"] = nc.dram_tensor("logits", [TC, E], F32, kind="ExternalOutput").ap()

    with tile.TileContext(nc) as tc:
        _emit(tc, nc, dr)

    nc.compile()
    return nc


def _emit(tc, nc, dr):
    from contextlib import ExitStack

    with ExitStack() as ctx:
        consts = ctx.enter_context(tc.tile_pool(name="consts", bufs=1))
        xin = ctx.enter_context(tc.tile_pool(name="xin", bufs=2))
        xts = ctx.enter_context(tc.tile_pool(name="xts", bufs=1))
        route = ctx.enter_context(tc.tile_pool(name="route", bufs=1))
        wpool = ctx.enter_context(tc.tile_pool(name="wpool", bufs=2))
        hpool = ctx.enter_context(tc.tile_pool(name="hpool", bufs=2))
        acc = ctx.enter_context(tc.tile_pool(name="acc", bufs=1))
        opool = ctx.enter_context(tc.tile_pool(name="opool", bufs=2))
        pgu = ctx.enter_context(tc.tile_pool(name="pgu", bufs=2, space="PSUM"))
        pdown = ctx.enter_context(tc.tile_pool(name="pdown", bufs=2, space="PSUM"))
        ptr = ctx.enter_context(tc.tile_pool(name="ptr", bufs=2, space="PSUM"))

        # ---------------- constants
        identity = consts.tile([P, P], F32)
        make_identity(nc, identity)
        # sel[:, e, :] is an [E, P] stationary operand with sel[k, m] = (k == e);
        # sel_e.T @ gatesT broadcasts expert e's gate row to all 128 partitions.
        sel = consts.tile([E, E, P], F32)
        for e in range(E):
            nc.vector.tensor_copy(
                sel[:, e, :], identity[:E, e : e + 1].to_broadcast((E, P))
            )
        wr_sb = consts.tile([P, KD, E], F32)
        nc.sync.dma_start(wr_sb[:], dr["wr"].rearrange("(kd p) e -> p kd e", p=P))

        # ---------------- load x, transpose to feature-major, cast to bf16
        xT_f32 = xts.tile([P, KD, TC], F32)
        xT_bf16 = xts.tile([P, KD, TC], BF16)
        for tt in range(TT):
            x_tm = xin.tile([P, D], F32, tag="x_tm")
            nc.sync.dma_start(x_tm[:], dr["x"][tt * P : (tt + 1) * P, :])
            for half in range(2):
                pt = ptr.tile([P, 512], F32, tag="ptr")
                for j in range(4):
                    kd = half * 4 + j
                    nc.tensor.transpose(
                        pt[:, j * P : (j + 1) * P],
                        x_tm[:, kd * P : (kd + 1) * P],
                        identity,
                    )
                dst_f = xT_f32[:, half * 4 : (half + 1) * 4, tt * P : (tt + 1) * P]
                dst_b = xT_bf16[:, half * 4 : (half + 1) * 4, tt * P : (tt + 1) * P]
                src = pt.rearrange("p (j q) -> p j q", j=4)
                nc.vector.tensor_copy(dst_f, src)
                nc.scalar.copy(dst_b, src)

        # ---------------- router (fp32): logitsT[e, t] then token-major math
        plog = ptr.tile([P, 512], F32, tag="ptr")
        for kd in range(KD):
            nc.tensor.matmul(
                plog[:E, :TC],
                wr_sb[:, kd, :],
                xT_f32[:, kd, :],
                start=(kd == 0),
                stop=(kd == KD - 1),
            )
        logitsT = route.tile([E, TC], F32)
        nc.vector.tensor_copy(logitsT[:], plog[:E, :TC])

        # transpose logitsT -> token-major [P, TT, E]
        logits_tm = route.tile([P, TT, E], F32)
        pl2 = ptr.tile([P, 512], F32, tag="ptr")
        for tt in range(TT):
            nc.tensor.transpose(
                pl2[:, tt * E : (tt + 1) * E],
                logitsT[:, tt * P : (tt + 1) * P],
                identity[:E, :E],
            )
        nc.vector.tensor_copy(
            logits_tm[:], pl2[:, : TT * E].rearrange("p (tt e) -> p tt e", tt=TT)
        )
        nc.sync.dma_start(
            dr["logits"].rearrange("(tt p) e -> p tt e", p=P), logits_tm[:]
        )

        # ---------------- top-2 + softmax gates (token-major)
        m1 = route.tile([P, TT], F32)
        nc.vector.tensor_reduce(m1[:], logits_tm[:], mybir.AxisListType.X, mybir.AluOpType.max)
        eq1 = route.tile([P, TT, E], F32)
        nc.vector.tensor_tensor(
            eq1[:], logits_tm[:], m1[:, :, None].to_broadcast((P, TT, E)),
            mybir.AluOpType.is_equal,
        )
        masked = route.tile([P, TT, E], F32)
        nc.vector.scalar_tensor_tensor(
            masked[:], eq1[:], -1e9, logits_tm[:],
            mybir.AluOpType.mult, mybir.AluOpType.add,
        )
        m2 = route.tile([P, TT], F32)
        nc.vector.tensor_reduce(m2[:], masked[:], mybir.AxisListType.X, mybir.AluOpType.max)
        eq2 = route.tile([P, TT, E], F32)
        nc.vector.tensor_tensor(
            eq2[:], masked[:], m2[:, :, None].to_broadcast((P, TT, E)),
            mybir.AluOpType.is_equal,
        )
        diff = route.tile([P, TT], F32)
        nc.vector.tensor_sub(diff[:], m1[:], m2[:])
        w1 = route.tile([P, TT], F32)
        nc.scalar.activation(w1[:], diff[:], mybir.ActivationFunctionType.Sigmoid)
        w2 = route.tile([P, TT], F32)
        nc.scalar.mul(w2[:], w1[:], -1.0)
        nc.vector.tensor_scalar_add(w2[:], w2[:], 1.0)

        gates_tm = route.tile([P, TT, E], F32)
        g1t = route.tile([P, TT, E], F32)
        nc.vector.tensor_mul(g1t[:], eq1[:], w1[:, :, None].to_broadcast((P, TT, E)))
        nc.vector.tensor_mul(gates_tm[:], eq2[:], w2[:, :, None].to_broadcast((P, TT, E)))
        nc.vector.tensor_add(gates_tm[:], gates_tm[:], g1t[:])

        # transpose gates -> [E, TC], then broadcast each expert row to 128 parts
        gatesT = route.tile([E, TC], F32)
        for tt in range(TT):
            pg2 = ptr.tile([P, 512], F32, tag="ptr")
            nc.tensor.transpose(
                pg2[:E, tt * P : (tt + 1) * P],
                gates_tm[:, tt, :],
                identity,
            )
            nc.vector.tensor_copy(
                gatesT[:, tt * P : (tt + 1) * P], pg2[:E, tt * P : (tt + 1) * P]
            )
        bcast = route.tile([P, E, TC], F32)
        for e in range(E):
            pb = ptr.tile([P, 512], F32, tag="ptr")
            nc.tensor.matmul(pb[:, :TC], sel[:, e, :], gatesT[:], start=True, stop=True)
            nc.vector.tensor_copy(bcast[:, e, :], pb[:, :TC])

        # ---------------- routed experts (dense, masked by gates)
        ydt = acc.tile([P, DT, TC], F32)
        for e in range(E):
            wg_sb = wpool.tile([P, KD, I], BF16, tag="wg")
            wu_sb = wpool.tile([P, KD, I], BF16, tag="wu")
            wd_sb = wpool.tile([P, IT, D], BF16, tag="wd")
            nc.sync.dma_start(
                wg_sb[:], dr["wg"][e].rearrange("(kd p) i -> p kd i", p=P)
            )
            nc.sync.dma_start(
                wu_sb[:], dr["wu"][e].rearrange("(kd p) i -> p kd i", p=P)
            )
            nc.sync.dma_start(
                wd_sb[:], dr["wd"][e].rearrange("(it p) d -> p it d", p=P)
            )
            h_e = hpool.tile([P, IT, TC], BF16, tag="h")
            for it in range(IT):
                pg = pgu.tile([P, TC], F32, tag="pg")
                for kd in range(KD):
                    nc.tensor.matmul(
                        pg[:],
                        wg_sb[:, kd, it * P : (it + 1) * P],
                        xT_bf16[:, kd, :],
                        start=(kd == 0),
                        stop=(kd == KD - 1),
                    )
                pu = pgu.tile([P, TC], F32, tag="pu")
                for kd in range(KD):
                    nc.tensor.matmul(
                        pu[:],
                        wu_sb[:, kd, it * P : (it + 1) * P],
                        xT_bf16[:, kd, :],
                        start=(kd == 0),
                        stop=(kd == KD - 1),
                    )
                sg = hpool.tile([P, TC], F32, tag="sg")
                nc.scalar.activation(sg[:], pg[:], mybir.ActivationFunctionType.Silu)
                h1 = hpool.tile([P, TC], F32, tag="h1")
                nc.vector.tensor_mul(h1[:], sg[:], pu[:])
                nc.vector.tensor_mul(h_e[:, it, :], h1[:], bcast[:, e, :])
            for dt_ in range(DT):
                pd = pdown.tile([P, TC], F32, tag="pd")
                for it in range(IT):
                    nc.tensor.matmul(
                        pd[:],
                        wd_sb[:, it, dt_ * P : (dt_ + 1) * P],
                        h_e[:, it, :],
                        start=(it == 0),
                        stop=(it == IT - 1),
                    )
                if e == 0:
                    nc.vector.tensor_copy(ydt[:, dt_, :], pd[:])
                else:
                    nc.vector.tensor_add(ydt[:, dt_, :], ydt[:, dt_, :], pd[:])

        # ---------------- shared expert, in 4 si-groups of 4 si-tiles
        for q in range(SQ):
            wgs_sb = wpool.tile([P, KD, 512], BF16, tag="wgs")
            wus_sb = wpool.tile([P, KD, 512], BF16, tag="wus")
            wds_sb = wpool.tile([P, 4, D], BF16, tag="wds")
            nc.sync.dma_start(
                wgs_sb[:],
                dr["wgs"][:, q * 512 : (q + 1) * 512].rearrange(
                    "(kd p) i -> p kd i", p=P
                ),
            )
            nc.sync.dma_start(
                wus_sb[:],
                dr["wus"][:, q * 512 : (q + 1) * 512].rearrange(
                    "(kd p) i -> p kd i", p=P
                ),
            )
            nc.sync.dma_start(
                wds_sb[:],
                dr["wds"][q * 512 : (q + 1) * 512, :].rearrange(
                    "(st p) d -> p st d", p=P
                ),
            )
            hs = hpool.tile([P, 4, TC], BF16, tag="hs")
            for j in range(4):
                pg = pgu.tile([P, TC], F32, tag="pg")
                for kd in range(KD):
                    nc.tensor.matmul(
                        pg[:],
                        wgs_sb[:, kd, j * P : (j + 1) * P],
                        xT_bf16[:, kd, :],
                        start=(kd == 0),
                        stop=(kd == KD - 1),
                    )
                pu = pgu.tile([P, TC], F32, tag="pu")
                for kd in range(KD):
                    nc.tensor.matmul(
                        pu[:],
                        wus_sb[:, kd, j * P : (j + 1) * P],
                        xT_bf16[:, kd, :],
                        start=(kd == 0),
                        stop=(kd == KD - 1),
                    )
                sg = hpool.tile([P, TC], F32, tag="sg")
                nc.scalar.activation(sg[:], pg[:], mybir.ActivationFunctionType.Silu)
                nc.vector.tensor_mul(hs[:, j, :], sg[:], pu[:])
            for dt_ in range(DT):
                pd = pdown.tile([P, TC], F32, tag="pd")
                for j in range(4):
                    nc.tensor.matmul(
                        pd[:],
                        wds_sb[:, j, dt_ * P : (dt_ + 1) * P],
                        hs[:, j, :],
                        start=(j == 0),
                        stop=(j == 3),
                    )
                nc.vector.tensor_add(ydt[:, dt_, :], ydt[:, dt_, :], pd[:])

        # ---------------- transpose back to token-major and store
        for tt in range(TT):
            out_tm = opool.tile([P, D], F32, tag="out_tm")
            for half in range(2):
                pt = ptr.tile([P, 512], F32, tag="ptr")
                for j in range(4):
                    dt_ = half * 4 + j
                    nc.tensor.transpose(
                        pt[:, j * P : (j + 1) * P],
                        ydt[:, dt_, tt * P : (tt + 1) * P],
                        identity,
                    )
                nc.vector.tensor_copy(
                    out_tm[:, half * 512 : (half + 1) * 512], pt[:]
                )
            nc.sync.dma_start(dr["y"][tt * P : (tt + 1) * P, :], out_tm[:])


def _get_compiled():
    global _COMPILED
    if _COMPILED is None:
        _COMPILED = _build_program()
    return _COMPILED


def kernel(hidden_states, Wr, Wg, Wu, Wd, Wg_s, Wu_s, Wd_s):
    global LAST_RESULT
    x = np.ascontiguousarray(np.asarray(hidden_states, np.float32).reshape(T, D))
    wr = np.ascontiguousarray(np.asarray(Wr, np.float32))
    bf = ml_dtypes.bfloat16
    wg = np.ascontiguousarray(np.asarray(Wg).astype(bf))
    wu = np.ascontiguousarray(np.asarray(Wu).astype(bf))
    wd = np.ascontiguousarray(np.asarray(Wd).astype(bf))
    wgs = np.ascontiguousarray(np.asarray(Wg_s).astype(bf))
    wus = np.ascontiguousarray(np.asarray(Wu_s).astype(bf))
    wds = np.ascontiguousarray(np.asarray(Wd_s).astype(bf))

    in_maps = []
    for c in range(NCORES):
        in_maps.append(
            {
                "x": x[c * TC : (c + 1) * TC],
                "wr": wr,
                "wg": wg,
                "wu": wu,
                "wd": wd,
                "wgs": wgs,
                "wus": wus,
                "wds": wds,
            }
        )

    nc = _get_compiled()
    res = run_bass_kernel_spmd(
        nc,
        in_maps,
        list(range(NCORES)),
        trace=bool(os.environ.get("MOE_TRACE")),
    )
    LAST_RESULT = res
    y = np.concatenate([res.results[c]["y"] for c in range(NCORES)], axis=0)
    logits = np.concatenate(
        [res.results[c]["logits"] for c in range(NCORES)], axis=0
    )
    return y.reshape(B, S, D), logits
